# revision 1
# baseline (speedup 1.0000x reference)
"""Paged-attention decode (GQA + ALiBi) Bass kernel for 8 Trainium2 cores.

Problem shape (hardcoded):
  query        [64, 32, 128] f32
  key_cache    [8192, 8, 16, 128] f32
  value_cache  [8192, 8, 16, 128] f32
  block_tables [64, 128] i32
  seq_lens     [64] i32
  out          [64, 32, 128] f32

Sharding: data-parallel over sequences. 64 seqs -> 8 cores x 8 slots; seqs are
sorted by length and dealt snake-wise so every core's slot j has a similar
chunk count. One SPMD program is value-specialized only on the per-slot chunk
counts CNT[0..7] (max over cores); everything else (block ids, ALiBi
rel/mask rows, q) flows in as per-core input data, so a single NEFF runs on
all 8 cores.

Per (slot, chunk of 128 positions = 8 KV blocks):
  - indirect-DMA gather of 8 K blocks and 8 V blocks (64KB contiguous each)
    into SBUF laid out [ (block,l) partitions, (kvh, d) free ]
  - per kv head: PE transpose K -> K^T [d, l], then matmul
    scoresT[l, g] += K^T(stationary) @ qT(moving, N=4)
  - one bias matmul adds ALiBi slope*rel + mask via a rank-2 update
  - ACT exp -> probs [l=128, 32 heads]
  - per kv head: outT[d, g] += V(natural, stationary) @ probs(moving, N=4)
    accumulated in PSUM across chunks; denominator via ones-vector matmul
Epilogue per slot: PE transpose outT -> [h, d], multiply by 1/denom, DMA out.

Softmax uses no max-subtraction: logits = q.k*scale + alibi <= ~10 here
(alibi <= 0, q.k*scale ~ N(0,1)), so exp never overflows; masked positions
get -1e30 -> exp == 0 exactly.
"""

import os
import numpy as np

S, H, KVH, G, D = 64, 32, 8, 4, 128
BS, L, NBLOCKS = 16, 2048, 8192
N_CORES = 8
CH = 128            # positions per chunk
BPC = CH // BS      # blocks per chunk
NEG = -1.0e30

_prog_cache = {}
LAST_NC = None      # for test harnesses: the last built Bass module


def _build_program(cnt):
    """Build the SPMD Bass program for per-slot chunk counts `cnt` (len 8)."""
    from contextlib import ExitStack

    import concourse.bass as bass
    import concourse.tile as tile
    from concourse import bacc, mybir
    from concourse.masks import make_identity

    f32 = mybir.dt.float32
    i32 = mybir.dt.int32
    tot = sum(cnt)
    cum = [0]
    for c in cnt:
        cum.append(cum[-1] + c)

    nc = bacc.Bacc(
        "TRN2",
        target_bir_lowering=False,
        debug=False,
        enable_asserts=False,
        num_devices=N_CORES,
    )
    kc_d = nc.dram_tensor("kc", [NBLOCKS, KVH, BS, D], f32, kind="ExternalInput")
    vc_d = nc.dram_tensor("vc", [NBLOCKS, KVH, BS, D], f32, kind="ExternalInput")
    qT_d = nc.dram_tensor("qT", [D, 8 * H], f32, kind="ExternalInput")
    ko_d = nc.dram_tensor("ko", [128, tot], i32, kind="ExternalInput")
    rm_d = nc.dram_tensor("rm", [2, tot * CH], f32, kind="ExternalInput")
    so_d = nc.dram_tensor("so", [2, H], f32, kind="ExternalInput")
    out_d = nc.dram_tensor("out", [8, H, D], f32, kind="ExternalOutput")

    with ExitStack() as ctx:
        tc = ctx.enter_context(tile.TileContext(nc))
        const = ctx.enter_context(tc.tile_pool(name="const", bufs=1))
        kvp = ctx.enter_context(tc.tile_pool(name="kv", bufs=4))
        ktp = ctx.enter_context(tc.tile_pool(name="kt", bufs=3))
        prp = ctx.enter_context(tc.tile_pool(name="pr", bufs=3))
        epp = ctx.enter_context(tc.tile_pool(name="ep", bufs=2))
        psT = ctx.enter_context(tc.tile_pool(name="psT", bufs=3, space="PSUM"))
        psS = ctx.enter_context(tc.tile_pool(name="psS", bufs=2, space="PSUM"))
        psA = ctx.enter_context(tc.tile_pool(name="psA", bufs=2, space="PSUM"))
        psD = ctx.enter_context(tc.tile_pool(name="psD", bufs=1, space="PSUM"))

        ident = const.tile([128, 128], f32)
        make_identity(nc, ident[:])
        ones = const.tile([128, 1], f32)
        nc.gpsimd.memset(ones[:], 1.0)
        qT_s = const.tile([D, 8 * H], f32)
        nc.sync.dma_start(qT_s[:], qT_d.ap())
        ko_s = const.tile([128, tot], i32)
        nc.sync.dma_start(ko_s[:], ko_d.ap())
        rm_s = const.tile([2, tot * CH], f32)
        nc.sync.dma_start(rm_s[:], rm_d.ap())
        so_s = const.tile([2, H], f32)
        nc.sync.dma_start(so_s[:], so_d.ap())

        for j in range(8):
            acc = psA.tile([128, H], f32)  # outT[d, h] accumulator
            den = psD.tile([H, 1], f32)  # softmax denominator per head
            for t in range(cnt[j]):
                ct = cum[j] + t
                ksb = kvp.tile([128, KVH * D], f32, tag="k")
                vsb = kvp.tile([128, KVH * D], f32, tag="v")
                for csb, cd in ((ksb, kc_d), (vsb, vc_d)):
                    # canonical indirect1d gather: one index per partition
                    # row, each fetching one contiguous [D] row. in_ viewed
                    # [NBLOCKS*KVH*BS, D] => coef = D; host index encodes
                    # block*BS + l; element_offset picks the kv head.
                    for h in range(KVH):
                        nc.gpsimd.indirect_dma_start(
                            out=csb[:, h * D : (h + 1) * D],
                            out_offset=None,
                            in_=cd.ap().rearrange("b h l d -> (b h l) d"),
                            in_offset=bass.IndirectOffsetOnAxis(
                                ap=ko_s[:, ct : ct + 1], axis=0
                            ),
                            element_offset=h * BS * D,
                        )
                kt = ktp.tile([128, KVH * D], f32)
                for h in range(KVH):
                    tp = psT.tile([128, 128], f32)
                    nc.tensor.transpose(
                        tp[:], ksb[:, h * D : (h + 1) * D], ident[:]
                    )
                    nc.vector.tensor_copy(kt[:, h * D : (h + 1) * D], tp[:])
                # bias first: one start=True writer for the whole PSUM zero
                # region (start marks the full 2KB region pending-zero, so it
                # must be unique and first); QK matmuls then accumulate.
                sc = psS.tile([128, H], f32)
                nc.tensor.matmul(
                    sc[:],
                    lhsT=rm_s[:, ct * CH : (ct + 1) * CH],
                    rhs=so_s[:],
                    start=True,
                    stop=False,
                )
                for h in range(KVH):
                    nc.tensor.matmul(
                        sc[:, G * h : G * (h + 1)],
                        lhsT=kt[:, h * D : (h + 1) * D],
                        rhs=qT_s[:, j * H + G * h : j * H + G * (h + 1)],
                        start=False,
                        stop=h == KVH - 1,
                    )
                pr = prp.tile([128, H], f32)
                nc.scalar.activation(
                    pr[:], sc[:], mybir.ActivationFunctionType.Exp
                )
                first, last = t == 0, t == cnt[j] - 1
                for h in range(KVH):
                    nc.tensor.matmul(
                        acc[:, G * h : G * (h + 1)],
                        lhsT=vsb[:, h * D : (h + 1) * D],
                        rhs=pr[:, G * h : G * (h + 1)],
                        start=first and h == 0,
                        stop=last and h == KVH - 1,
                    )
                nc.tensor.matmul(
                    den[:],
                    lhsT=pr[:],
                    rhs=ones[:],
                    start=first,
                    stop=last,
                )
            # epilogue: outT [d, h] -> [h, d], divide by denom, store
            oT = epp.tile([128, H], f32, tag="oT")
            nc.vector.tensor_copy(oT[:], acc[:])
            rec = epp.tile([H, 1], f32, tag="rec")
            nc.vector.reciprocal(rec[:], den[:])
            of = psT.tile([H, 128], f32, tag="tp")
            nc.tensor.transpose(of[:], oT[:], ident[:])
            osb = epp.tile([H, 128], f32, tag="osb")
            nc.vector.tensor_scalar_mul(osb[:], of[:], rec[:])
            nc.sync.dma_start(out_d.ap()[j], osb[:])

    nc.compile()
    return nc


def _prep(
    query,
    key_cache,
    value_cache,
    scale,
    block_tables,
    seq_lens,
    alibi_slopes,
):
    q = np.asarray(query, dtype=np.float32)
    kc = np.ascontiguousarray(np.asarray(key_cache, dtype=np.float32))
    vc = np.ascontiguousarray(np.asarray(value_cache, dtype=np.float32))
    bt = np.asarray(block_tables, dtype=np.int32)
    sl = np.asarray(seq_lens, dtype=np.int64)
    slope = np.asarray(alibi_slopes, dtype=np.float32)
    sc_f = float(np.asarray(scale))

    nch = np.maximum(1, -(-sl // CH))  # ceil, >= 1
    order = np.argsort(-nch, kind="stable")
    assign = np.empty((8, N_CORES), np.int64)  # [slot, core] -> seq idx
    for j in range(8):
        grp = order[j * 8 : (j + 1) * 8]
        assign[j] = grp if j % 2 == 0 else grp[::-1]
    cnt = tuple(int(nch[assign[j]].max()) for j in range(8))
    tot = sum(cnt)
    cum = np.concatenate([[0], np.cumsum(cnt)])

    # per-core input tensors
    so = np.stack([slope, np.ones(H, np.float32)]).astype(np.float32)  # [2, 32]
    in_maps = []
    for c in range(N_CORES):
        qT = np.zeros((D, 8 * H), np.float32)
        ko = np.zeros((128, tot), np.int32)
        rm = np.zeros((2, tot * CH), np.float32)
        for j in range(8):
            s = int(assign[j, c])
            qT[:, j * H : (j + 1) * H] = (q[s] * sc_f).T  # [128, 32]
            n = int(cnt[j])
            # per-partition-row gather indices: partition p of chunk t maps
            # to block bt[s, t*BPC + p//BS], row l = p % BS; the device view
            # is [NBLOCKS*KVH*BS, D] rows, head offset added on device.
            # (padded chunks use whatever the block table holds -- valid
            # ids, contributions masked to zero)
            blk = bt[s, : n * BPC].reshape(n, BPC).astype(np.int64)  # [n, 8]
            rows = np.repeat(blk * KVH * BS, BS, axis=1) + np.tile(
                np.arange(BS), BPC
            )
            ko[:, cum[j] : cum[j] + n] = rows.T.astype(np.int32)
            ln = int(sl[s])
            pos = np.arange(n * CH)
            valid = pos < ln
            rel = np.where(valid, pos - (ln - 1), 0).astype(np.float32)
            msk = np.where(valid, 0.0, NEG).astype(np.float32)
            rm[0, cum[j] * CH : (cum[j] + n) * CH] = rel
            rm[1, cum[j] * CH : (cum[j] + n) * CH] = msk
        in_maps.append(
            {"kc": kc, "vc": vc, "qT": qT, "ko": ko, "rm": rm, "so": so}
        )
    return cnt, assign, in_maps


def kernel(
    query,
    key_cache,
    value_cache,
    num_kv_heads,
    scale,
    block_tables,
    seq_lens,
    block_size,
    max_seq_len,
    alibi_slopes,
):
    global LAST_NC
    from concourse.bass_utils import run_bass_kernel_spmd

    cnt, assign, in_maps = _prep(
        query, key_cache, value_cache, scale, block_tables, seq_lens, alibi_slopes
    )

    if cnt not in _prog_cache:
        _prog_cache[cnt] = _build_program(cnt)
    nc = _prog_cache[cnt]
    LAST_NC = nc

    res = run_bass_kernel_spmd(nc, in_maps, core_ids=list(range(N_CORES)))

    out = np.empty((S, H, D), np.float32)
    for c in range(N_CORES):
        o = res.results[c]["out"]  # [8, 32, 128]
        for j in range(8):
            out[int(assign[j, c])] = o[j]
    return out



# revision 5
# speedup vs baseline: 2.7254x; 2.7254x over previous
"""Paged-attention decode (GQA + ALiBi) Bass kernel for 8 Trainium2 cores.

Flat chunk-unit design, v2.

Problem shape (hardcoded):
  query        [64, 32, 128] f32
  key_cache    [8192, 8, 16, 128] f32   (block b, kv head h, pos l, d)
  value_cache  [8192, 8, 16, 128] f32
  block_tables [64, 128] i32
  seq_lens     [64] i32
  out          [64, 32, 128] f32

Work is decomposed into flat "units": one unit = 128 contiguous positions
(8 KV blocks) of one sequence, all 8 KV heads.  Total units are dealt
evenly across the 8 cores (M units per core, padded with fully-masked
dummies).  Each unit produces PARTIAL results (pv = V^T @ exp(scores),
den = sum exp(scores)) that the host combines per sequence:
  out[s] = (sum_u pv_u) / (sum_u den_u).

Device per pair of units (one 1MB K gather + one 1MB V gather):
  - indirect DMA with ONE index per partition (the only HW-supported
    form): partition p = (u, h, b) fetches the contiguous 8KB block-head
    slice [16 l, 128 d] of (block b of unit u, head h).
  - K path: 16 PE transposes (l-slices) -> kt sbuf [d, (j,u,h,b)];
    per (u,h) the QK lhsT is a strided AP over kt giving K^T [d, 128
    positions in permuted order pi: row i=(j*8+b) <-> l=b*16+j].
  - V path: same 16 transposes -> vd sbuf [d, (j,u,h,b)]; then per
    (u,h) a second transpose of a strided vd AP -> V[pi(l), d] -> vt.
  - scores: rank-2 ALiBi bias matmul (rel,mask rows x slope,ones) then
    8 QK matmuls accumulate; ACT exp -> probs; den via ones matmul and
    8 PV matmuls into one PSUM bank; single DMA out of [128, 33].

Softmax uses no max-subtraction: logits = q.k*scale + alibi <= ~10,
masked positions get -1e30 -> exp == 0 exactly.
"""

import numpy as np

S, H, KVH, G, D = 64, 32, 8, 4, 128
BS, L, NBLOCKS = 16, 2048, 8192
N_CORES = 8
CH = 128            # positions per unit
BPU = CH // BS      # blocks per unit (8)
NEG = -1.0e30

_prog_cache = {}
LAST_NC = None

# pool depths (tunable; see prof_sim sweeps)
TUNE = {"kv": 3, "kt": 2, "vd": 2, "vt": 2, "pr": 2, "po": 2, "pipe": 1}


def _build_program(M):
    """SPMD Bass program processing M units (M even) per core."""
    from contextlib import ExitStack

    import concourse.bass as bass
    import concourse.tile as tile
    from concourse import bacc, mybir
    f32 = mybir.dt.float32
    f32r = mybir.dt.float32r
    i32 = mybir.dt.int32
    NPAIR = M // 2
    PIPE = TUNE["pipe"]

    nc = bacc.Bacc(
        "TRN2",
        target_bir_lowering=False,
        debug=False,
        enable_asserts=False,
        num_devices=N_CORES,
    )
    kc_d = nc.dram_tensor("kc", [NBLOCKS, KVH, BS, D], f32r, kind="ExternalInput")
    vc_d = nc.dram_tensor("vc", [NBLOCKS, KVH, BS, D], f32r, kind="ExternalInput")
    qT_d = nc.dram_tensor("qT", [D, M * H], f32r, kind="ExternalInput")
    ko_d = nc.dram_tensor("ko", [128, NPAIR], i32, kind="ExternalInput")
    rm_d = nc.dram_tensor("rm", [2, M * CH], f32, kind="ExternalInput")
    so_d = nc.dram_tensor("so", [2, H], f32, kind="ExternalInput")
    id_d = nc.dram_tensor("idm", [128, 128], f32r, kind="ExternalInput")
    o_d = nc.dram_tensor("o", [M, 128, H + 1], f32, kind="ExternalOutput")

    kv_view_k = kc_d.ap().rearrange("b h l d -> (b h) (l d)")
    kv_view_v = vc_d.ap().rearrange("b h l d -> (b h) (l d)")

    with ExitStack() as ctx:
        tc = ctx.enter_context(tile.TileContext(nc))
        const = ctx.enter_context(tc.tile_pool(name="const", bufs=1))
        kvp = ctx.enter_context(tc.tile_pool(name="kv", bufs=TUNE["kv"]))
        ktp = ctx.enter_context(tc.tile_pool(name="kt", bufs=TUNE["kt"]))
        vdp = ctx.enter_context(tc.tile_pool(name="vd", bufs=TUNE["vd"]))
        vtp = ctx.enter_context(tc.tile_pool(name="vt", bufs=TUNE["vt"]))
        prp = ctx.enter_context(tc.tile_pool(name="pr", bufs=TUNE["pr"]))
        pop = ctx.enter_context(tc.tile_pool(name="po", bufs=TUNE["po"]))
        psT = ctx.enter_context(tc.tile_pool(name="psT", bufs=2, space="PSUM"))
        psV2 = ctx.enter_context(tc.tile_pool(name="psV2", bufs=2, space="PSUM"))
        psS = ctx.enter_context(tc.tile_pool(name="psS", bufs=1, space="PSUM"))
        psO = ctx.enter_context(tc.tile_pool(name="psO", bufs=1, space="PSUM"))

        ident = const.tile([128, 128], f32r)
        nc.sync.dma_start(ident[:], id_d.ap())
        ones = const.tile([128, 1], f32)
        nc.gpsimd.memset(ones[:], 1.0)
        qT_s = const.tile([D, M * H], f32r)
        nc.sync.dma_start(qT_s[:], qT_d.ap())
        ko_s = const.tile([128, NPAIR], i32)
        nc.sync.dma_start(ko_s[:], ko_d.ap())
        rm_s = const.tile([2, M * CH], f32)
        nc.sync.dma_start(rm_s[:], rm_d.ap())
        so_s = const.tile([2, H], f32)
        nc.sync.dma_start(so_s[:], so_d.ap())

        def front(p_i):
            """Gathers + l-slice transposes + permute copies for one pair."""
            # pair tile: partition p=(u,h,b), free = one 8KB block-head slice
            ksb = kvp.tile([128, BS * D], f32r, tag="k")
            vsb = kvp.tile([128, BS * D], f32r, tag="v")
            for csb, cv in ((ksb, kv_view_k), (vsb, kv_view_v)):
                nc.gpsimd.indirect_dma_start(
                    out=csb[:],
                    out_offset=None,
                    in_=cv,
                    in_offset=bass.IndirectOffsetOnAxis(
                        ap=ko_s[:, p_i : p_i + 1], axis=0
                    ),
                    element_offset=0,
                )
            # l-slice transposes: Bk/Bv [p=(u,h,b), (l,d)] -> [d, (j, p)].
            # The psum->sbuf copies permute columns so that per (u,h) the
            # 128 K^T / V^T columns are CONTIGUOUS in kt/vd (walrus requires
            # matmul weight APs to have a single free dim):
            #   kt col = (u*8+h)*128 + j*8 + b,   j = half*8 + jj
            kt = ktp.tile([128, 16 * 128], f32r)
            vd = vdp.tile([128, 16 * 128], f32r)
            kt6 = kt[:].rearrange(
                "p (uu h hf jj b) -> p uu hf jj h b", uu=2, h=8, hf=2, jj=8
            )
            vd6 = vd[:].rearrange(
                "p (uu h hf jj b) -> p uu hf jj h b", uu=2, h=8, hf=2, jj=8
            )
            for half in range(2):
                for src, dst6, eng in ((ksb, kt6, "dve"), (vsb, vd6, "act")):
                    pt = psT.tile([128, 1024], f32r, tag="pt")
                    for jj in range(8):
                        j = half * 8 + jj
                        nc.tensor.matmul(
                            pt[:, jj * 128 : (jj + 1) * 128],
                            lhsT=src[:, j * 128 : (j + 1) * 128],
                            rhs=ident[:],
                            is_transpose=True,
                            start=jj % 4 == 0,
                            stop=jj % 4 == 3,
                        )
                    pt5 = pt[:].rearrange(
                        "p (jj uu h b) -> p uu jj h b", jj=8, uu=2, h=8
                    )
                    for u in range(2):
                        if eng == "dve":
                            nc.vector.tensor_copy(dst6[:, u, half], pt5[:, u])
                        else:
                            nc.scalar.activation(
                                dst6[:, u, half], pt5[:, u],
                                mybir.ActivationFunctionType.Copy,
                            )
            return kt, vd

        def back(p_i, kt, vd):
            """V2 transposes, scores, softmax, PV/den, output DMA."""
            for u in range(2):
                g_u = 2 * p_i + u  # global unit index on this core
                # V second transpose: per (u,h) contiguous vd slice; two
                # half-size psum tiles (1 bank each, double buffered)
                vt = vtp.tile([128, 1024], f32)
                for hg in range(2):
                    pv2 = psV2.tile([128, 512], f32r, tag="pv2")
                    for hh in range(4):
                        h = hg * 4 + hh
                        c0 = (u * 8 + h) * 128
                        nc.tensor.matmul(
                            pv2[:, hh * 128 : (hh + 1) * 128],
                            lhsT=vd[:, c0 : c0 + 128],
                            rhs=ident[:],
                            is_transpose=True,
                            start=hh == 0,
                            stop=hh == 3,
                        )
                    vsl = slice(hg * 512, (hg + 1) * 512)
                    if (2 * u + hg) % 2 == 0:
                        nc.scalar.activation(
                            vt[:, vsl], pv2[:],
                            mybir.ActivationFunctionType.Copy,
                        )
                    else:
                        nc.vector.tensor_copy(vt[:, vsl], pv2[:])
                # scores: bias (rank-2: slope*rel + mask) then 8 QK matmuls
                sc = psS.tile([128, H], f32, tag="sc")
                nc.tensor.matmul(
                    sc[:],
                    lhsT=rm_s[:, g_u * CH : (g_u + 1) * CH],
                    rhs=so_s[:],
                    start=True,
                    stop=False,
                )
                for h in range(KVH):
                    c0 = (u * 8 + h) * 128
                    nc.tensor.matmul(
                        sc[:, G * h : G * (h + 1)],
                        lhsT=kt[:, c0 : c0 + 128],
                        rhs=qT_s[:, g_u * H + G * h : g_u * H + G * (h + 1)],
                        start=False,
                        stop=h == KVH - 1,
                    )
                pr = prp.tile([128, H], f32)
                nc.scalar.activation(
                    pr[:], sc[:], mybir.ActivationFunctionType.Exp
                )
                po = psO.tile([128, H + 1], f32, tag="po")
                # pv h0 opens the accumulation group and pv h7 closes it
                # (start/stop bookkeeping is per-partition, so both must be
                # 128-partition matmuls); den (32 partitions) joins between.
                nc.tensor.matmul(
                    po[:, 0:G],
                    lhsT=vt[:, 0:128],
                    rhs=pr[:, 0:G],
                    start=True,
                    stop=False,
                )
                nc.tensor.matmul(
                    po[0:H, H : H + 1],
                    lhsT=pr[:],
                    rhs=ones[:],
                    start=False,
                    stop=False,
                )
                for h in range(1, KVH):
                    nc.tensor.matmul(
                        po[:, G * h : G * (h + 1)],
                        lhsT=vt[:, h * 128 : (h + 1) * 128],
                        rhs=pr[:, G * h : G * (h + 1)],
                        start=False,
                        stop=h == KVH - 1,
                    )
                ob = pop.tile([128, H + 1], f32)
                nc.vector.memset(ob[:, H : H + 1], 0.0)
                nc.vector.tensor_copy(ob[:, 0:H], po[:, 0:H])
                nc.vector.tensor_copy(ob[0:H, H : H + 1], po[0:H, H : H + 1])
                nc.sync.dma_start(o_d.ap()[g_u], ob[:])

        # software pipeline: emit fronts PIPE pairs ahead of backs so the
        # PE stream reaches next-pair transposes before stalling on pair
        # i's exp-dependent matmuls (keeps the gather DMAs fed).
        from collections import deque
        q = deque()
        for p_i in range(min(PIPE, NPAIR)):
            q.append(front(p_i))
        for p_i in range(NPAIR):
            if p_i + PIPE < NPAIR:
                q.append(front(p_i + PIPE))
            back(p_i, *q.popleft())

    nc.compile()
    return nc


def _prep(query, key_cache, value_cache, scale, block_tables, seq_lens,
          alibi_slopes):
    q = np.asarray(query, dtype=np.float32)
    kc = np.ascontiguousarray(np.asarray(key_cache, dtype=np.float32))
    vc = np.ascontiguousarray(np.asarray(value_cache, dtype=np.float32))
    bt = np.asarray(block_tables, dtype=np.int64)
    sl = np.asarray(seq_lens, dtype=np.int64)
    slope = np.asarray(alibi_slopes, dtype=np.float32)
    sc_f = float(np.asarray(scale))

    nch = np.maximum(1, -(-sl // CH))
    units = [(s, t) for s in range(S) for t in range(int(nch[s]))]
    total = len(units)
    M = -(-total // N_CORES)
    if M % 2:
        M += 1
    units += [(-1, 0)] * (N_CORES * M - total)

    qs = (q * sc_f).astype(np.float32)  # [S, H, D]

    in_maps = []
    unit_map = []  # [core][u] -> seq or -1
    so = np.stack([slope, np.ones(H, np.float32)]).astype(np.float32)
    pp = np.arange(128)
    u_of_p, h_of_p, b_of_p = pp // 64, (pp % 64) // 8, pp % 8
    for c in range(N_CORES):
        mine = units[c * M : (c + 1) * M]
        unit_map.append([s for s, _ in mine])
        qT = np.zeros((D, M * H), np.float32)
        ko = np.zeros((128, M // 2), np.int32)
        rm = np.empty((2, M * CH), np.float32)
        rm[0] = 0.0
        rm[1] = NEG
        for ui, (s, t) in enumerate(mine):
            if s < 0:
                continue  # pad: ko 0, rel 0, mask NEG
            qT[:, ui * H : (ui + 1) * H] = qs[s].T
            pi, uu = ui // 2, ui % 2
            sel = u_of_p == uu
            blk = bt[s, t * BPU + b_of_p[sel]]
            ko[sel, pi] = (blk * KVH + h_of_p[sel]).astype(np.int32)
            # score row i <-> position l = (i%8)*16 + i//8
            i = np.arange(CH)
            pos = t * CH + (i % 8) * 16 + i // 8
            valid = pos < sl[s]
            rm[0, ui * CH : (ui + 1) * CH] = np.where(valid, pos - (sl[s] - 1), 0)
            rm[1, ui * CH : (ui + 1) * CH] = np.where(valid, 0.0, NEG)
        in_maps.append({"kc": kc, "vc": vc, "qT": qT, "ko": ko, "rm": rm,
                        "so": so, "idm": np.eye(128, dtype=np.float32)})
    return M, unit_map, in_maps


def kernel(query, key_cache, value_cache, num_kv_heads, scale, block_tables,
           seq_lens, block_size, max_seq_len, alibi_slopes):
    global LAST_NC
    from concourse.bass_utils import run_bass_kernel_spmd

    M, unit_map, in_maps = _prep(
        query, key_cache, value_cache, scale, block_tables, seq_lens,
        alibi_slopes
    )

    if M not in _prog_cache:
        _prog_cache[M] = _build_program(M)
    nc = _prog_cache[M]
    LAST_NC = nc

    res = run_bass_kernel_spmd(nc, in_maps, core_ids=list(range(N_CORES)))

    acc = np.zeros((S, D, H), np.float64)
    den = np.zeros((S, H), np.float64)
    for c in range(N_CORES):
        o = np.asarray(res.results[c]["o"], dtype=np.float64)  # [M, 128, H+1]
        for ui, s in enumerate(unit_map[c]):
            if s < 0:
                continue
            acc[s] += o[ui, :, :H]
            den[s] += o[ui, :H, H]
    out = acc / den[:, None, :]
    return np.ascontiguousarray(out.transpose(0, 2, 1)).astype(np.float32)
